# revision 2
# baseline (speedup 1.0000x reference)
"""BlockAttnRes Trainium2 kernel, v3 — dot products on the PE.

Two host-prepared layouts of the same V (bf16):
  v  [TSH, NB*D]            partition=t tiles; feeds ACT ss + output pass
  vt [NT*128, 16*NB*128]    partition=d_lo tiles (d = dh*128+dl); free is
                            (dh, n, th) so the dh-slice is contiguous.
Dots: out[kind, (n,th)] = sum_dl qw_k[dh*128+dl] * V[dl, (n,th)] accumulated
over dh in PSUM — stationary is a [128,2] qw slice, so the whole dot costs
one PE stream of V.  Dot results [2,(n,th)] are PE-transposed per block to
[th,2] and gathered as [128,(n,kind)] for a t-partitioned softmax.
rsqrt(u) on DVE by Newton (u=ss/D+eps is within ~15% of 1), so ACT uses a
single table set {Square, Exp, Copy}.
Output: per (kind,tile) unit either PE (diag matmul, PSUM) or DVE
(tensor_scalar by attn column at 4x + pairwise add tree).
"""

import math
import os
import numpy as np

N_CORES = 8
NB = 8
B, T, D = 2, 4096, 2048
BT = B * T
TSH = BT // N_CORES
P = 128
NT = TSH // P
NDH = D // P          # 16 d_hi slices
EPS = 1e-6
MM_N = 512
PE_UNITS = int(os.environ.get("K3_PE_UNITS", "5"))   # of 16 (kind,tile) units

_CACHE = {}


def _build():
    from contextlib import ExitStack

    import concourse.bass as bass
    import concourse.tile as tile
    from concourse import bacc, mybir

    f32 = mybir.dt.float32
    bf16 = mybir.dt.bfloat16
    f16 = mybir.dt.float16
    Alu = mybir.AluOpType
    Act = mybir.ActivationFunctionType

    nc = bacc.Bacc("TRN2", target_bir_lowering=False, debug=False,
                   num_devices=N_CORES)

    v = nc.dram_tensor("v", [TSH, NB * D], bf16, kind="ExternalInput").ap()
    vt = nc.dram_tensor("vt", [NT * P, NDH * NB * P], bf16,
                        kind="ExternalInput").ap()
    qws_d = nc.dram_tensor("qws", [P, 2 * NDH], bf16,
                           kind="ExternalInput").ap()
    ident_d = nc.dram_tensor("ident", [P, P], bf16, kind="ExternalInput").ap()
    id2_d = nc.dram_tensor("id2", [2, 2], f16, kind="ExternalInput").ap()
    o = nc.dram_tensor("o", [TSH, 2 * D], f16, kind="ExternalOutput").ap()

    pe_set = {int((j + 0.5) * 16 / PE_UNITS) for j in range(PE_UNITS)} \
        if PE_UNITS > 0 else set()

    with tile.TileContext(nc) as tc, ExitStack() as ctx:
        const_p = ctx.enter_context(tc.tile_pool(name="const", bufs=1))
        xb_p = ctx.enter_context(tc.tile_pool(name="xb", bufs=3))
        vtb_p = ctx.enter_context(tc.tile_pool(name="vtb", bufs=1))
        scr_p = ctx.enter_context(tc.tile_pool(name="scr", bufs=2))
        stat_p = ctx.enter_context(tc.tile_pool(name="stat", bufs=3))
        diag_p = ctx.enter_context(tc.tile_pool(name="diag", bufs=4))
        out_p = ctx.enter_context(tc.tile_pool(name="out", bufs=2))
        psum_p = ctx.enter_context(
            tc.tile_pool(name="psum", bufs=1, space=bass.MemorySpace.PSUM))
        psd_p = ctx.enter_context(
            tc.tile_pool(name="psd", bufs=2, space=bass.MemorySpace.PSUM))
        pst_p = ctx.enter_context(
            tc.tile_pool(name="pst", bufs=2, space=bass.MemorySpace.PSUM))

        qws = const_p.tile([P, 2 * NDH], bf16)
        ident = const_p.tile([P, P], bf16)
        id2 = const_p.tile([2, 2], f16)
        nc.sync.dma_start(out=qws[:], in_=qws_d[:])
        nc.sync.dma_start(out=ident[:], in_=ident_d[:])
        nc.sync.dma_start(out=id2[:], in_=id2_d[:])

        for i in range(NT):
            t0 = i * P
            vtt = vtb_p.tile([P, NDH * NB * P], bf16, tag="vtt")
            nc.scalar.dma_start(out=vtt[:], in_=vt[t0:t0 + P, :])
            xt = xb_p.tile([P, NB * D], bf16, tag="xt")
            nc.sync.dma_start(out=xt[:], in_=v[t0:t0 + P, :])

            def xn(n):
                return xt[:, n * D:(n + 1) * D]

            # ---- dots on PE: psum [2, 512] per 4-block group -------------
            dvf = stat_p.tile([2, NB * P], f16, tag="dvf")
            for g in range(2):
                psd = psd_p.tile([2, 512], f32, tag="psd")
                for dh in range(NDH):
                    off = dh * NB * P + g * 512
                    nc.tensor.matmul(
                        psd[:], qws[:, 2 * dh:2 * dh + 2],
                        vtt[:, off:off + 512],
                        start=(dh == 0), stop=(dh == NDH - 1))
                nc.vector.tensor_copy(dvf[:, g * 512:(g + 1) * 512], psd[:])

            # ---- transpose dv to [th, (n, kind)] interleaved -------------
            dvt = stat_p.tile([P, 2 * NB], f16, tag="dvt")
            pst = pst_p.tile([P, 2 * NB], f16, tag="pst")
            for n in range(NB):
                nc.tensor.transpose(pst[:, 2 * n:2 * n + 2],
                                    dvf[:, n * P:(n + 1) * P], id2[:])
            nc.vector.tensor_copy(dvt[:], pst[:])

            # ---- ss on ACT; w = rsqrt(ss/D+eps) via DVE Newton -----------
            ss = stat_p.tile([P, NB], f32, tag="ss")
            sqo = scr_p.tile([P, D], bf16, tag="sqo")
            for n in range(NB):
                nc.scalar.activation(sqo[:], xn(n), Act.Square,
                                     accum_out=ss[:, n:n + 1])
            u = stat_p.tile([P, NB], f32, tag="u")
            nc.scalar.activation(u[:], ss[:], Act.Copy, bias=EPS,
                                 scale=1.0 / D)
            w = stat_p.tile([P, NB], f32, tag="w")
            nc.vector.tensor_scalar(w[:], u[:], -0.5, 1.5, Alu.mult, Alu.add)
            for it in range(2):
                t1 = stat_p.tile([P, NB], f32, tag="t1")
                nc.vector.tensor_tensor(out=t1[:], in0=w[:], in1=w[:],
                                        op=Alu.mult)
                t2 = stat_p.tile([P, NB], f32, tag="t2")
                nc.vector.tensor_tensor(out=t2[:], in0=t1[:], in1=u[:],
                                        op=Alu.mult)
                t3 = stat_p.tile([P, NB], f32, tag="t3")
                nc.vector.tensor_scalar(t3[:], t2[:], -0.5, 1.5, Alu.mult,
                                        Alu.add)
                w2 = stat_p.tile([P, NB], f32, tag="w")
                nc.vector.tensor_tensor(out=w2[:], in0=w[:], in1=t3[:],
                                        op=Alu.mult)
                w = w2

            # ---- per kind: softmax + output ------------------------------
            ob = out_p.tile([P, 2 * D], f16, tag="ob")
            for kidx, odoff in ((0, 0), (1, D)):
                lg = stat_p.tile([P, NB], f32, tag=f"lg{kidx}")
                nc.vector.tensor_tensor(
                    out=lg[:], in0=dvt[:, kidx:2 * NB:2],
                    in1=w[:], op=Alu.mult)
                e = stat_p.tile([P, NB], f32, tag=f"e{kidx}")
                se = stat_p.tile([P, 1], f32, tag=f"se{kidx}")
                nc.scalar.activation(e[:], lg[:], Act.Exp, accum_out=se[:])
                r = stat_p.tile([P, 1], f32, tag=f"r{kidx}")
                nc.vector.reciprocal(r[:], se[:])
                en = stat_p.tile([P, NB], f32, tag=f"en{kidx}")
                nc.vector.tensor_scalar(en[:], e[:], r[:, 0:1], None,
                                        Alu.mult)

                unit = i * 2 + kidx
                if (unit % 16) in pe_set:
                    # PE route: diag matmuls into PSUM, ACT copy out
                    diags = []
                    for n in range(NB):
                        dg = diag_p.tile([P, P], bf16, tag="dg")
                        nc.vector.tensor_scalar(dg[:], ident[:],
                                                en[:, n:n + 1], None,
                                                Alu.mult)
                        diags.append(dg)
                    ps = psum_p.tile([P, D], f32, tag="ps")
                    for n in range(NB):
                        for j in range(D // MM_N):
                            nc.tensor.matmul(
                                ps[:, j * MM_N:(j + 1) * MM_N],
                                diags[n][:],
                                xn(n)[:, j * MM_N:(j + 1) * MM_N],
                                start=(n == 0), stop=(n == NB - 1))
                    nc.scalar.activation(ob[:, odoff:odoff + D], ps[:],
                                         Act.Copy)
                else:
                    # DVE route: attn-scaled terms (TS 4x) + pairwise adds
                    pairs = []
                    for np_ in range(NB // 2):
                        ta = scr_p.tile([P, D], bf16, tag="tn0")
                        nc.vector.tensor_scalar(ta[:], xn(2 * np_),
                                                en[:, 2 * np_:2 * np_ + 1],
                                                None, Alu.mult)
                        tb = scr_p.tile([P, D], bf16, tag="tn1")
                        nc.vector.tensor_scalar(tb[:], xn(2 * np_ + 1),
                                                en[:, 2 * np_ + 1:2 * np_ + 2],
                                                None, Alu.mult)
                        s = scr_p.tile([P, D], bf16, tag=f"s{np_ % 2}")
                        nc.vector.tensor_tensor(out=s[:], in0=ta[:],
                                                in1=tb[:], op=Alu.add)
                        pairs.append(s)
                    f1 = scr_p.tile([P, D], bf16, tag="tn0")
                    nc.vector.tensor_tensor(out=f1[:], in0=pairs[0][:],
                                            in1=pairs[1][:], op=Alu.add)
                    f2 = scr_p.tile([P, D], bf16, tag="tn1")
                    nc.vector.tensor_tensor(out=f2[:], in0=pairs[2][:],
                                            in1=pairs[3][:], op=Alu.add)
                    nc.vector.tensor_tensor(out=ob[:, odoff:odoff + D],
                                            in0=f1[:], in1=f2[:],
                                            op=Alu.add)

            nc.gpsimd.dma_start(out=o[t0:t0 + P, :], in_=ob[:])

    nc.compile()
    return nc


def _get_nc():
    if "nc" not in _CACHE:
        _CACHE["nc"] = _build()
    return _CACHE["nc"]


def kernel(blocks, partial_block, pseudo_query_attn, pseudo_query_mlp,
           norm_weight_attn, norm_weight_mlp):
    import ml_dtypes
    from concourse.bass_utils import run_bass_kernel_spmd

    nc = _get_nc()
    bf16 = ml_dtypes.bfloat16
    f32 = np.float32

    V = np.concatenate(
        [np.asarray(blocks, f32).reshape(NB - 1, BT, D),
         np.asarray(partial_block, f32).reshape(1, BT, D)], axis=0)

    scale = 1.0 / math.sqrt(D)
    qwa = (np.asarray(pseudo_query_attn, f32)
           * np.asarray(norm_weight_attn, f32) * scale)
    qwm = (np.asarray(pseudo_query_mlp, f32)
           * np.asarray(norm_weight_mlp, f32) * scale)
    # qws[dl, 2*dh+k] = qw_k[dh*128+dl]
    qws = np.empty((P, 2 * NDH), dtype=f32)
    qws[:, 0::2] = qwa.reshape(NDH, P).T
    qws[:, 1::2] = qwm.reshape(NDH, P).T
    qws = qws.astype(bf16)
    ident = np.eye(P, dtype=f32).astype(bf16)
    id2 = np.eye(2, dtype=np.float16)

    in_maps = []
    for c in range(N_CORES):
        sl = V[:, c * TSH:(c + 1) * TSH, :].transpose(1, 0, 2)  # [t, n, d]
        sl = np.ascontiguousarray(sl).astype(bf16)
        vrow = sl.reshape(TSH, NB * D)
        # vt: [i, th, n, dh, dl] -> (i, dl, dh, n, th)
        vtt = sl.reshape(NT, P, NB, NDH, P).transpose(0, 4, 3, 2, 1)
        vtt = np.ascontiguousarray(vtt).reshape(NT * P, NDH * NB * P)
        in_maps.append({"v": vrow, "vt": vtt, "qws": qws, "ident": ident,
                        "id2": id2})

    kw = {}
    if os.environ.get("KERNEL_TRACE"):
        kw = {"trace": True, "tmpdir": os.environ.get("KERNEL_TRACE_DIR")}
    res = run_bass_kernel_spmd(nc, in_maps, list(range(N_CORES)), **kw)
    _CACHE["last_result"] = res

    outs = [res.results[c]["o"].reshape(TSH, 2, D) for c in range(N_CORES)]
    full = np.concatenate(outs, axis=0)
    ha = full[:, 0, :].astype(f32).reshape(B, T, D)
    hm = full[:, 1, :].astype(f32).reshape(B, T, D)
    return (ha, hm)


# revision 3
# speedup vs baseline: 1.0799x; 1.0799x over previous
"""BlockAttnRes Trainium2 kernel.

Computes, for V = stack([*blocks, partial_block]) and two (q, w) pairs:
  K = rmsnorm(V, w); logits = (q.K)/sqrt(D); attn = softmax_n; out = attn.V
using q.K = rsqrt(mean(V^2)+eps) * ((q*w/sqrt(D)) . V), so K is never built.
Sharding: data parallel over flattened (B,T), 1024 rows/core, params
replicated.

Two host-prepared layouts of the same V (bf16) per core:
  v  [TSH, NB*D]            partition=t tiles; feeds ACT ss + output pass
  vt [NT*128, 16*NB*128]    partition=d_lo tiles (d = dh*128+dl); free is
                            (dh, n, th) so the dh-slice is contiguous.
Dots: out[kind, (n,th)] = sum_dl qw_k[dh*128+dl] * V[dl, (n,th)] accumulated
over dh in PSUM — stationary is a [128,2] qw slice, so both dot products
cost one PE stream of V (they do NOT fit the 1x-rate DVE reduce path, which
was the original bottleneck).  Dot results [2,(n,th)] are PE-transposed per
block to [th,2] and gathered as [128,(n,kind)] for a t-partitioned softmax.
rsqrt(u) on DVE by Newton (u=ss/D+eps is within ~15% of 1), so ACT uses a
single activation-table set {Square, Exp, Copy} — no table switching.
Output: per (kind,tile) unit either PE (diag(attn) matmul, PSUM accum) or
DVE (tensor_scalar by attn column + pairwise add tree), balanced PE_UNITS
of 16 on the PE.
"""

import math
import os
import numpy as np

N_CORES = 8
NB = 8
B, T, D = 2, 4096, 2048
BT = B * T
TSH = BT // N_CORES
P = 128
NT = TSH // P
NDH = D // P          # 16 d_hi slices
EPS = 1e-6
MM_N = 512
PE_UNITS = int(os.environ.get("K3_PE_UNITS", "5"))   # of 16 (kind,tile) units

_CACHE = {}


def _build():
    from contextlib import ExitStack

    import concourse.bass as bass
    import concourse.tile as tile
    from concourse import bacc, mybir

    f32 = mybir.dt.float32
    bf16 = mybir.dt.bfloat16
    f16 = mybir.dt.float16
    Alu = mybir.AluOpType
    Act = mybir.ActivationFunctionType

    nc = bacc.Bacc("TRN2", target_bir_lowering=False, debug=False,
                   num_devices=N_CORES)

    v = nc.dram_tensor("v", [TSH, NB * D], bf16, kind="ExternalInput").ap()
    vt = nc.dram_tensor("vt", [NT * P, NDH * NB * P], bf16,
                        kind="ExternalInput").ap()
    qws_d = nc.dram_tensor("qws", [P, 2 * NDH], bf16,
                           kind="ExternalInput").ap()
    ident_d = nc.dram_tensor("ident", [P, P], bf16, kind="ExternalInput").ap()
    id2_d = nc.dram_tensor("id2", [2, 2], f16, kind="ExternalInput").ap()
    o = nc.dram_tensor("o", [TSH, 2 * D], f16, kind="ExternalOutput").ap()

    pe_set = {int((j + 0.5) * 16 / PE_UNITS) for j in range(PE_UNITS)} \
        if PE_UNITS > 0 else set()

    with tile.TileContext(nc) as tc, ExitStack() as ctx:
        const_p = ctx.enter_context(tc.tile_pool(name="const", bufs=1))
        xb_p = ctx.enter_context(tc.tile_pool(name="xb", bufs=3))
        vtb_p = ctx.enter_context(tc.tile_pool(name="vtb", bufs=1))
        scr_p = ctx.enter_context(tc.tile_pool(name="scr", bufs=2))
        stat_p = ctx.enter_context(tc.tile_pool(name="stat", bufs=3))
        diag_p = ctx.enter_context(tc.tile_pool(name="diag", bufs=4))
        out_p = ctx.enter_context(tc.tile_pool(name="out", bufs=2))
        psum_p = ctx.enter_context(
            tc.tile_pool(name="psum", bufs=1, space=bass.MemorySpace.PSUM))
        psd_p = ctx.enter_context(
            tc.tile_pool(name="psd", bufs=2, space=bass.MemorySpace.PSUM))
        pst_p = ctx.enter_context(
            tc.tile_pool(name="pst", bufs=2, space=bass.MemorySpace.PSUM))

        qws = const_p.tile([P, 2 * NDH], bf16)
        ident = const_p.tile([P, P], bf16)
        id2 = const_p.tile([2, 2], f16)
        nc.sync.dma_start(out=qws[:], in_=qws_d[:])
        nc.sync.dma_start(out=ident[:], in_=ident_d[:])
        nc.sync.dma_start(out=id2[:], in_=id2_d[:])

        for i in range(NT):
            t0 = i * P
            vtt = vtb_p.tile([P, NDH * NB * P], bf16, tag="vtt")
            nc.scalar.dma_start(out=vtt[:], in_=vt[t0:t0 + P, :])
            xt = xb_p.tile([P, NB * D], bf16, tag="xt")
            nc.sync.dma_start(out=xt[:], in_=v[t0:t0 + P, :])

            def xn(n):
                return xt[:, n * D:(n + 1) * D]

            # ---- dots on PE: psum [2, 512] per 4-block group -------------
            dvf = stat_p.tile([2, NB * P], f16, tag="dvf")
            for g in range(2):
                psd = psd_p.tile([2, 512], f32, tag="psd")
                for dh in range(NDH):
                    off = dh * NB * P + g * 512
                    nc.tensor.matmul(
                        psd[:], qws[:, 2 * dh:2 * dh + 2],
                        vtt[:, off:off + 512],
                        start=(dh == 0), stop=(dh == NDH - 1))
                nc.vector.tensor_copy(dvf[:, g * 512:(g + 1) * 512], psd[:])

            # ---- transpose dv to [th, (n, kind)] interleaved -------------
            dvt = stat_p.tile([P, 2 * NB], f16, tag="dvt")
            pst = pst_p.tile([P, 2 * NB], f16, tag="pst")
            for n in range(NB):
                nc.tensor.transpose(pst[:, 2 * n:2 * n + 2],
                                    dvf[:, n * P:(n + 1) * P], id2[:])
            nc.vector.tensor_copy(dvt[:], pst[:])

            # ---- ss on ACT; w = rsqrt(ss/D+eps) via DVE Newton -----------
            ss = stat_p.tile([P, NB], f32, tag="ss")
            sqo = scr_p.tile([P, D], bf16, tag="sqo")
            for n in range(NB):
                nc.scalar.activation(sqo[:], xn(n), Act.Square,
                                     accum_out=ss[:, n:n + 1])
            u = stat_p.tile([P, NB], f32, tag="u")
            nc.scalar.activation(u[:], ss[:], Act.Copy, bias=EPS,
                                 scale=1.0 / D)
            w = stat_p.tile([P, NB], f32, tag="w")
            nc.vector.tensor_scalar(w[:], u[:], -0.5, 1.5, Alu.mult, Alu.add)
            for it in range(2):
                t1 = stat_p.tile([P, NB], f32, tag="t1")
                nc.vector.tensor_tensor(out=t1[:], in0=w[:], in1=w[:],
                                        op=Alu.mult)
                t2 = stat_p.tile([P, NB], f32, tag="t2")
                nc.vector.tensor_tensor(out=t2[:], in0=t1[:], in1=u[:],
                                        op=Alu.mult)
                t3 = stat_p.tile([P, NB], f32, tag="t3")
                nc.vector.tensor_scalar(t3[:], t2[:], -0.5, 1.5, Alu.mult,
                                        Alu.add)
                w2 = stat_p.tile([P, NB], f32, tag="w")
                nc.vector.tensor_tensor(out=w2[:], in0=w[:], in1=t3[:],
                                        op=Alu.mult)
                w = w2

            # ---- per kind: softmax + output ------------------------------
            ob = out_p.tile([P, 2 * D], f16, tag="ob")
            for kidx, odoff in ((0, 0), (1, D)):
                lg = stat_p.tile([P, NB], f32, tag=f"lg{kidx}")
                nc.vector.tensor_tensor(
                    out=lg[:], in0=dvt[:, kidx:2 * NB:2],
                    in1=w[:], op=Alu.mult)
                e = stat_p.tile([P, NB], f32, tag=f"e{kidx}")
                se = stat_p.tile([P, 1], f32, tag=f"se{kidx}")
                nc.scalar.activation(e[:], lg[:], Act.Exp, accum_out=se[:])
                r = stat_p.tile([P, 1], f32, tag=f"r{kidx}")
                nc.vector.reciprocal(r[:], se[:])
                en = stat_p.tile([P, NB], f32, tag=f"en{kidx}")
                nc.vector.tensor_scalar(en[:], e[:], r[:, 0:1], None,
                                        Alu.mult)

                unit = i * 2 + kidx
                if (unit % 16) in pe_set:
                    # PE route: diag matmuls into PSUM, ACT copy out
                    diags = []
                    for n in range(NB):
                        dg = diag_p.tile([P, P], bf16, tag="dg")
                        nc.vector.tensor_scalar(dg[:], ident[:],
                                                en[:, n:n + 1], None,
                                                Alu.mult)
                        diags.append(dg)
                    ps = psum_p.tile([P, D], f32, tag="ps")
                    for n in range(NB):
                        for j in range(D // MM_N):
                            nc.tensor.matmul(
                                ps[:, j * MM_N:(j + 1) * MM_N],
                                diags[n][:],
                                xn(n)[:, j * MM_N:(j + 1) * MM_N],
                                start=(n == 0), stop=(n == NB - 1))
                    nc.scalar.activation(ob[:, odoff:odoff + D], ps[:],
                                         Act.Copy)
                else:
                    # DVE route: attn-scaled terms (TS 4x) + pairwise adds
                    pairs = []
                    for np_ in range(NB // 2):
                        ta = scr_p.tile([P, D], bf16, tag="tn0")
                        nc.vector.tensor_scalar(ta[:], xn(2 * np_),
                                                en[:, 2 * np_:2 * np_ + 1],
                                                None, Alu.mult)
                        tb = scr_p.tile([P, D], bf16, tag="tn1")
                        nc.vector.tensor_scalar(tb[:], xn(2 * np_ + 1),
                                                en[:, 2 * np_ + 1:2 * np_ + 2],
                                                None, Alu.mult)
                        s = scr_p.tile([P, D], bf16, tag=f"s{np_ % 2}")
                        nc.vector.tensor_tensor(out=s[:], in0=ta[:],
                                                in1=tb[:], op=Alu.add)
                        pairs.append(s)
                    f1 = scr_p.tile([P, D], bf16, tag="tn0")
                    nc.vector.tensor_tensor(out=f1[:], in0=pairs[0][:],
                                            in1=pairs[1][:], op=Alu.add)
                    f2 = scr_p.tile([P, D], bf16, tag="tn1")
                    nc.vector.tensor_tensor(out=f2[:], in0=pairs[2][:],
                                            in1=pairs[3][:], op=Alu.add)
                    nc.vector.tensor_tensor(out=ob[:, odoff:odoff + D],
                                            in0=f1[:], in1=f2[:],
                                            op=Alu.add)

            nc.gpsimd.dma_start(out=o[t0:t0 + P, :], in_=ob[:])

    nc.compile()
    return nc


def _get_nc():
    if "nc" not in _CACHE:
        _CACHE["nc"] = _build()
    return _CACHE["nc"]


def kernel(blocks, partial_block, pseudo_query_attn, pseudo_query_mlp,
           norm_weight_attn, norm_weight_mlp):
    import ml_dtypes
    from concourse.bass_utils import run_bass_kernel_spmd

    nc = _get_nc()
    bf16 = ml_dtypes.bfloat16
    f32 = np.float32

    V = np.concatenate(
        [np.asarray(blocks, f32).reshape(NB - 1, BT, D),
         np.asarray(partial_block, f32).reshape(1, BT, D)], axis=0)

    scale = 1.0 / math.sqrt(D)
    qwa = (np.asarray(pseudo_query_attn, f32)
           * np.asarray(norm_weight_attn, f32) * scale)
    qwm = (np.asarray(pseudo_query_mlp, f32)
           * np.asarray(norm_weight_mlp, f32) * scale)
    # qws[dl, 2*dh+k] = qw_k[dh*128+dl]
    qws = np.empty((P, 2 * NDH), dtype=f32)
    qws[:, 0::2] = qwa.reshape(NDH, P).T
    qws[:, 1::2] = qwm.reshape(NDH, P).T
    qws = qws.astype(bf16)
    ident = np.eye(P, dtype=f32).astype(bf16)
    id2 = np.eye(2, dtype=np.float16)

    in_maps = []
    for c in range(N_CORES):
        sl = V[:, c * TSH:(c + 1) * TSH, :].transpose(1, 0, 2)  # [t, n, d]
        sl = np.ascontiguousarray(sl).astype(bf16)
        vrow = sl.reshape(TSH, NB * D)
        # vt: [i, th, n, dh, dl] -> (i, dl, dh, n, th)
        vtt = sl.reshape(NT, P, NB, NDH, P).transpose(0, 4, 3, 2, 1)
        vtt = np.ascontiguousarray(vtt).reshape(NT * P, NDH * NB * P)
        in_maps.append({"v": vrow, "vt": vtt, "qws": qws, "ident": ident,
                        "id2": id2})

    kw = {}
    if os.environ.get("KERNEL_TRACE"):
        kw = {"trace": True, "tmpdir": os.environ.get("KERNEL_TRACE_DIR")}
    res = run_bass_kernel_spmd(nc, in_maps, list(range(N_CORES)), **kw)
    _CACHE["last_result"] = res

    outs = [res.results[c]["o"].reshape(TSH, 2, D) for c in range(N_CORES)]
    full = np.concatenate(outs, axis=0)
    ha = full[:, 0, :].astype(f32).reshape(B, T, D)
    hm = full[:, 1, :].astype(f32).reshape(B, T, D)
    return (ha, hm)
